# revision 35
# baseline (speedup 1.0000x reference)
"""Trainium2 Bass kernel for nn_DiscreteLSTMActor.

Architecture (hardcoded shapes): T=128, B=256, OBS=1024, FEAT=512, A=15,
D=528 (padded to 640), 4D=2112 (padded to 2560).

Sharding: data-parallel over B across 8 NeuronCores (32 rows each).
Per core:
  - encoder matmul (weights-stationary, obs pre-transposed on host, bf16)
  - x-gates for all T precomputed per 8-step block (cuDNN-style)
  - sequential T-scan: weights-stationary W_hh matmuls (gates kept
    transposed: gate-dim on partitions, batch on free); x-gates+bias are
    seeded into PSUM with identity matmuls so W_hh accumulates on top;
    elementwise on ACT/DVE/Pool, episode-reset masks via broadcast APs
  - two independent scan chains (layer 1 of block b, layer 2 of block
    b-1) interleaved so elementwise hides under the other chain's
    matmuls; next block's encoder/x-gates woven in as PE gap fillers
  - policy/baseline heads + argmax fused per block
"""
import sys
sys.path.insert(0, "/opt/trn_rl_repo")

import numpy as np
import ml_dtypes

import concourse.bacc as bacc
import concourse.mybir as mybir
from concourse.tile import TileContext
from concourse.bass_utils import run_bass_kernel_spmd

f32 = mybir.dt.float32
bf16 = mybir.dt.bfloat16
i32 = mybir.dt.int32
u32 = mybir.dt.uint32
AF = mybir.ActivationFunctionType
ALU = mybir.AluOpType
BF = ml_dtypes.bfloat16

T, B, OBS, FEAT, A = 128, 256, 1024, 512, 15
D, G = 528, 2112
Dp, Gp = 640, 2560          # padded: 5 and 20 chunks of 128
KC, MC = Dp // 128, Gp // 128
NC_ = 8
BL = B // NC_               # 32 batch rows per core
TB = T * BL                 # 4096
Tb = 8                      # timesteps per block
NBLK = T // Tb              # 8
BC = Tb * BL                # TB-columns per block
GATE_ORDER = (0, 1, 3, 2)   # device gate layout: i, f, o, g


def _build_nc():
    nc = bacc.Bacc("TRN2", target_bir_lowering=False, debug=False,
                   enable_asserts=True, num_devices=NC_)

    dram = {}
    def din(name, shape, dt):
        dram[name] = nc.dram_tensor(name, shape, dt, kind="ExternalInput").ap()
    def dout(name, shape, dt):
        dram[name] = nc.dram_tensor(name, shape, dt, kind="ExternalOutput").ap()

    din("obsT", [OBS, TB], bf16)
    din("rew", [TB], f32)
    din("act", [TB], i32)
    din("ndm", [(T + 1) * BL], f32)
    din("ones_row", [TB], bf16)
    din("enc_wT", [OBS, FEAT], bf16)
    din("enc_b", [FEAT], f32)
    for l in range(2):
        din(f"wihT{l}", [Dp, Gp], bf16)
        din(f"whhT{l}", [Dp, Gp], bf16)
    din("headWT", [Dp, 16], bf16)
    din("headb", [16], f32)
    dout("pol", [TB, A], f32)
    dout("bsl", [TB], f32)
    dout("aout", [TB], i32)

    with TileContext(nc) as tc:
        _emit(nc, tc, dram)
    nc.compile()
    return nc


def _emit(nc, tc, dram):
    import contextlib
    ctx = contextlib.ExitStack()
    with ctx:
        res = ctx.enter_context(tc.tile_pool(name="res", bufs=1))    # residents
        ps = ctx.enter_context(tc.tile_pool(name="ps", bufs=2, space="PSUM"))
        sbw = ctx.enter_context(tc.tile_pool(name="sbw", bufs=2))    # working
        obsp = ctx.enter_context(tc.tile_pool(name="obsp", bufs=8))  # 8 obs tiles live
        mrp = ctx.enter_context(tc.tile_pool(name="mrp", bufs=2))    # per-block masks

        # ---------------- resident tiles ----------------
        whh = [res.tile([128, KC * Gp], bf16, tag=f"whh{l}", name=f"whh{l}") for l in range(2)]
        wih = [res.tile([128, KC * Gp], bf16, tag=f"wih{l}", name=f"wih{l}") for l in range(2)]
        enc_sb = res.tile([128, 8 * FEAT], bf16, tag="encw")
        headW = res.tile([128, KC * 16], bf16, tag="headw")
        ch4 = res.tile([128, TB], bf16, tag="ch4")
        G1l = [res.tile([128, MC * BC], bf16, tag=f"gsb1_{i}", name=f"G1_{i}") for i in range(2)]
        G2 = res.tile([128, MC * BC], bf16, tag="gsb2")
        H1 = res.tile([128, KC * BC], bf16, tag="h1blk")
        H2 = res.tile([128, KC * BC], bf16, tag="h2blk")
        featl = [res.tile([128, 4 * BC], bf16, tag=f"feat{i}", name=f"feat{i}") for i in range(2)]
        ebias = res.tile([128, 4], f32, tag="ebias")
        hbias = res.tile([16, 1], f32, tag="hbias")
        iota_f = res.tile([128, 1], f32, tag="iotaf")
        ident = res.tile([16, 16], f32, tag="ident")
        ident128 = res.tile([128, 128], bf16, tag="ident128")
        ones_b = res.tile([1, 128], bf16, tag="onesb")
        ones_f = res.tile([1, 128], f32, tag="onesf")
        h_st = [res.tile([128, KC * BL], bf16, tag=f"h{l}st", name=f"h{l}st") for l in range(2)]
        c_st = [res.tile([128, KC * BL], f32, tag=f"c{l}st", name=f"c{l}st") for l in range(2)]

        # ---------------- one-time setup ----------------
        for l in range(2):
            w = dram[f"whhT{l}"]
            wi = dram[f"wihT{l}"]
            for k in range(KC):
                nc.scalar.dma_start(out=whh[l][:, k * Gp:(k + 1) * Gp],
                                    in_=w[k * 128:(k + 1) * 128, :])
                nc.scalar.dma_start(out=wih[l][:, k * Gp:(k + 1) * Gp],
                                    in_=wi[k * 128:(k + 1) * 128, :])
        for k in range(8):
            nc.sync.dma_start(out=enc_sb[:, k * FEAT:(k + 1) * FEAT],
                              in_=dram["enc_wT"][k * 128:(k + 1) * 128, :])
        for k in range(KC):
            nc.sync.dma_start(out=headW[:, k * 16:(k + 1) * 16],
                              in_=dram["headWT"][k * 128:(k + 1) * 128, :])
        nc.sync.dma_start(out=ebias[:],
                          in_=dram["enc_b"][:].rearrange("(c p) -> p c", p=128))
        nc.sync.dma_start(out=hbias[:],
                          in_=dram["headb"][:].rearrange("(p o) -> p o", o=1))

        iota_i = sbw.tile([128, 1], i32, tag="ioti")
        nc.gpsimd.iota(iota_i[:], pattern=[[0, 1]], base=-1, channel_multiplier=1)
        nc.vector.tensor_copy(iota_f[:], iota_i[:])
        id_row_i = sbw.tile([16, 16], i32, tag="idri")
        nc.gpsimd.iota(id_row_i[:], pattern=[[1, 16]], base=0, channel_multiplier=0)
        id_row_f = sbw.tile([16, 16], f32, tag="idrf")
        nc.vector.tensor_copy(id_row_f[:], id_row_i[:])
        id_col_i = sbw.tile([16, 1], i32, tag="idci")
        nc.gpsimd.iota(id_col_i[:], pattern=[[0, 1]], base=0, channel_multiplier=1)
        id_col_f = sbw.tile([16, 1], f32, tag="idcf")
        nc.vector.tensor_copy(id_col_f[:], id_col_i[:])
        nc.vector.tensor_scalar(ident[:], id_row_f[:], id_col_f[:, 0:1], None,
                                op0=ALU.is_equal)
        nc.vector.memset(ones_b[:], 1.0)
        nc.vector.memset(ones_f[:], 1.0)
        ir128 = sbw.tile([128, 128], i32, tag="ir128")
        nc.gpsimd.iota(ir128[:], pattern=[[1, 128]], base=0, channel_multiplier=0)
        ir128f = sbw.tile([128, 128], f32, tag="ir128f")
        nc.vector.tensor_copy(ir128f[:], ir128[:])
        ic128 = sbw.tile([128, 1], i32, tag="ic128")
        nc.gpsimd.iota(ic128[:], pattern=[[0, 1]], base=0, channel_multiplier=1)
        ic128f = sbw.tile([128, 1], f32, tag="ic128f")
        nc.vector.tensor_copy(ic128f[:], ic128[:])
        nc.vector.tensor_scalar(ident128[:], ir128f[:], ic128f[:, 0:1], None,
                                op0=ALU.is_equal)

        # scrub PSUM slots once (stale NaNs would poison pad-row elementwise)
        for i in range(3):
            sc1 = ps.tile([128, 640], f32, tag="gates", name=f"scrub_g{i}", bufs=3)
            nc.vector.memset(sc1[:], 0.0)
            if i < 1:
                sc2 = ps.tile([128, 4 * BC], f32, tag="mmbig", name=f"scrub_b{i}", bufs=1)
                nc.vector.memset(sc2[:], 0.0)

        # ch4 = [clipped_reward; one_hot(act); zeros] in transposed layout
        act_d = dram["act"][:].rearrange("(o a) -> o a", o=1)
        rew_d = dram["rew"][:].rearrange("(o a) -> o a", o=1)
        for j in range(TB // 512):
            cs = slice(j * 512, (j + 1) * 512)
            act_i = sbw.tile([1, 512], i32, tag="acti", name=f"acti{j}")
            nc.sync.dma_start(out=act_i[:], in_=act_d[:, cs])
            act_b = sbw.tile([1, 512], bf16, tag="actb", name=f"actb{j}")
            nc.vector.tensor_copy(act_b[:], act_i[:])
            arep = ps.tile([128, 512], f32, tag="mmbig", name=f"arep{j}", bufs=1)
            nc.tensor.matmul(arep[:], lhsT=ones_b[:], rhs=act_b[:],
                             start=True, stop=True)
            nc.vector.tensor_scalar(ch4[:, cs], arep[:],
                                    iota_f[:, 0:1], None, op0=ALU.is_equal)
            rew_row = sbw.tile([1, 512], f32, tag="rewr", name=f"rewr{j}")
            nc.sync.dma_start(out=rew_row[:], in_=rew_d[:, cs])
            nc.vector.tensor_scalar(ch4[0:1, cs], rew_row[:], 1.0, -1.0,
                                    op0=ALU.min, op1=ALU.max)

        nc.sync.dma_start(out=ch4[16:17, :],
                          in_=dram["ones_row"][:].rearrange("(o a) -> o a", o=1))
        for l in range(2):
            nc.vector.memset(h_st[l][:], 0.0)
            nc.vector.memset(c_st[l][:], 0.0)

        ndm_ap = dram["ndm"][:]

        # ---------------- phase helpers ----------------

        def enc_dma(b):
            col0 = b * BC
            obs_t = [obsp.tile([128, BC], bf16, tag="obst", name=f"obst{b}_{k}")
                     for k in range(8)]
            for k in range(8):
                nc.sync.dma_start(out=obs_t[k][:],
                                  in_=dram["obsT"][k * 128:(k + 1) * 128, col0:col0 + BC])
            return obs_t

        def enc_piece(b, obs_t, m, feat):
            pf = ps.tile([128, BC], f32, tag="mmbig", name=f"pf{b}_{m}", bufs=1)
            for k in range(8):
                nc.tensor.matmul(pf[:], lhsT=enc_sb[:, k * FEAT + m * 128: k * FEAT + (m + 1) * 128],
                                 rhs=obs_t[k][:], start=(k == 0), stop=(k == 7))
            nc.vector.tensor_scalar(feat[:, m * BC:(m + 1) * BC], pf[:],
                                    ebias[:, m:m + 1], 0.0,
                                    op0=ALU.add, op1=ALU.max)

        def xg_grp(l, b, wt, Gdst, feat, grp, evict="act"):
            col0 = b * BC
            if True:
                pg = ps.tile([128, 4 * BC], f32, tag="mmbig", name=f"pg{l}_{b}_{grp}", bufs=1)
                for ci in range(4):
                    c = grp * 4 + ci
                    mw = 16 if c % 5 == 4 else 128     # pad-tail chunks are thin
                    for k in range(KC):
                        if l == 0:
                            rhs = feat[:, k * BC:(k + 1) * BC] if k < 4 \
                                else ch4[:, col0:col0 + BC]
                        else:
                            rhs = H1[:, k * BC:(k + 1) * BC]
                        nc.tensor.matmul(pg[0:mw, ci * BC:(ci + 1) * BC],
                                         lhsT=wt[:, k * Gp + c * 128: k * Gp + c * 128 + mw],
                                         rhs=rhs, start=(k == 0), stop=(k == KC - 1))
                if evict == "act":
                    nc.scalar.copy(Gdst[:, grp * 4 * BC:(grp + 1) * 4 * BC], pg[:])
                else:
                    nc.vector.tensor_copy(Gdst[:, grp * 4 * BC:(grp + 1) * 4 * BC], pg[:])

        def xgates(l, b, wt, Gdst, feat=None):
            for grp in range(5):
                xg_grp(l, b, wt, Gdst, feat, grp)

        def mask_block(b):
            MW = (Tb + 1) * BL
            nd_row = sbw.tile([1, MW], f32, tag="ndr", name=f"ndr{b}")
            nc.sync.dma_start(out=nd_row[:],
                              in_=ndm_ap[b * Tb * BL:(b * Tb + Tb + 1) * BL]
                              .rearrange("(o a) -> o a", o=1))
            pm = ps.tile([128, 640], f32, tag="gates", name=f"pm{b}", bufs=3)
            nc.tensor.matmul(pm[:, 0:MW], lhsT=ones_f[:], rhs=nd_row[:],
                             start=True, stop=True)
            Mt = mrp.tile([128, MW], f32, tag="mrep", name=f"mrep{b}")
            nc.vector.tensor_copy(Mt[:], pm[:, 0:MW])
            return Mt

        def rec_step(l, b, t, Gsrc, Mt, Hout):
            """One layer-step. Gate layout: i [0:160) f [160:320) o [320:480) g [480:640)."""
            tagp = f"L{l}"
            Gv = Gsrc[:].rearrange("p (c s) -> p c s", c=MC)
            Hv = Hout[:].rearrange("p (k s) -> p k s", k=KC)
            Mv = Mt[:].rearrange("p (s c) -> p s c", s=Tb + 1)
            hs, cs_ = h_st[l], c_st[l]
            wl = whh[l]
            pgt = ps.tile([128, 640], f32, tag="gates", name=f"pgt{l}_{b}_{t}", bufs=3)
            # seed PSUM with the x-gates via identity matmuls, then accumulate W_hh
            for c in range(MC):
                # start=True clears has_written for the WHOLE bank - only the
                # first matmul touching each bank (cols 0 and 512) may clear
                nc.tensor.matmul(pgt[:, c * 32:(c + 1) * 32], lhsT=ident128[:],
                                 rhs=Gsrc[:, c * BC + t * 32: c * BC + (t + 1) * 32],
                                 start=(c in (0, 16)), stop=False,
                                 skip_group_check=True)
            for c in range(MC):
                mw = 16 if c % 5 == 4 else 128         # pad-tail chunks are thin
                for k in range(KC):
                    nc.tensor.matmul(pgt[0:mw, c * 32:(c + 1) * 32],
                                     lhsT=wl[:, k * Gp + c * 128: k * Gp + c * 128 + mw],
                                     rhs=hs[:, k * BL:(k + 1) * BL],
                                     start=False, stop=(k == KC - 1),
                                     skip_group_check=True)
            sg = sbw.tile([128, 640], f32, tag="sg" + tagp, name=f"sg{l}_{b}_{t}")
            nc.scalar.activation(sg[:, 0:320], pgt[:, 0:320], AF.Sigmoid)
            nc.scalar.activation(sg[:, 480:640], pgt[:, 480:640], AF.Tanh)
            nc.scalar.activation(sg[:, 320:480], pgt[:, 320:480], AF.Sigmoid)
            mt = Mv[:, t, :].rearrange("p (o c) -> p o c", o=1).broadcast_to([128, KC, BL])
            mt1 = Mv[:, t + 1, :].rearrange("p (o c) -> p o c", o=1).broadcast_to([128, KC, BL])
            r3 = lambda ap: ap.rearrange("p (k c) -> p k c", k=KC)
            hm1 = sbw.tile([128, 160], f32, tag="hm1" + tagp, name=f"hm1{l}_{b}_{t}")
            nc.gpsimd.tensor_mul(r3(hm1[:]), r3(sg[:, 320:480]), mt1)
            fm = sbw.tile([128, 160], f32, tag="fm" + tagp, name=f"fm{l}_{b}_{t}")
            nc.vector.tensor_mul(r3(fm[:]), r3(sg[:, 160:320]), mt)
            fc = sbw.tile([128, 160], f32, tag="fc" + tagp, name=f"fc{l}_{b}_{t}")
            nc.vector.tensor_mul(fc[:], fm[:], cs_[:])
            ig = sbw.tile([128, 160], f32, tag="ig" + tagp, name=f"ig{l}_{b}_{t}")
            nc.vector.tensor_mul(ig[:], sg[:, 0:160], sg[:, 480:640])
            nc.vector.tensor_add(cs_[:], fc[:], ig[:])
            tnc = sbw.tile([128, 160], f32, tag="tnc" + tagp, name=f"tnc{l}_{b}_{t}")
            nc.scalar.activation(tnc[:], cs_[:], AF.Tanh)
            nc.vector.tensor_mul(hs[:], hm1[:], tnc[:])
            nc.gpsimd.tensor_mul(Hv[:, :, t * 32:(t + 1) * 32],
                                 r3(sg[:, 320:480]), r3(tnc[:]))

        def heads(b):
            col0 = b * BC
            plg = ps.tile([16, BC], f32, tag="mmbig", name=f"plg{b}", bufs=1)
            for k in range(KC):
                nc.tensor.matmul(plg[:], lhsT=headW[:, k * 16:(k + 1) * 16],
                                 rhs=H2[:, k * BC:(k + 1) * BC],
                                 start=(k == 0), stop=(k == KC - 1))
            lgT = sbw.tile([16, BC], f32, tag="lgT", name=f"lgT{b}")
            nc.vector.tensor_scalar_add(lgT[:], plg[:], hbias[:, 0:1])
            for j in range(BC // 128):
                ptr = ps.tile([128, 16], f32, tag="mmbig", name=f"ptr{b}_{j}", bufs=1)
                nc.tensor.transpose(ptr[:], lgT[:, j * 128:(j + 1) * 128], ident[:])
                lg = sbw.tile([128, 16], f32, tag="lg", name=f"lg{b}_{j}")
                nc.vector.tensor_copy(lg[:], ptr[:])
                r0 = col0 + j * 128
                nc.sync.dma_start(out=dram["pol"][r0:r0 + 128, :], in_=lg[:, 0:A])
                nc.sync.dma_start(out=dram["bsl"][r0:r0 + 128].rearrange("(p o) -> p o", o=1),
                                  in_=lg[:, A:A + 1])
                mx = sbw.tile([128, 8], f32, tag="mx", name=f"mx{b}_{j}")
                nc.vector.max(mx[:], lg[:, 0:A])
                mi = sbw.tile([128, 8], u32, tag="mi", name=f"mi{b}_{j}")
                nc.vector.max_index(mi[:], mx[:], lg[:, 0:A])
                mi32 = sbw.tile([128, 1], i32, tag="mi32", name=f"mi32{b}_{j}")
                nc.vector.tensor_copy(mi32[:], mi[:, 0:1])
                nc.sync.dma_start(out=dram["aout"][r0:r0 + 128].rearrange("(p o) -> p o", o=1),
                                  in_=mi32[:])

        # ---------------- software-pipelined block loop ----------------
        # iteration b: xg2(b-1) burst, then interleaved rec2(b-1)/rec1(b)
        # steps with enc(b+1)/xg1(b+1) pieces woven in as PE gap fillers.
        obs_next = enc_dma(0)
        for m in range(4):
            enc_piece(0, obs_next, m, featl[0])
        xgates(0, 0, wih[0], G1l[0], featl[0])
        prev_M = None
        for b in range(NBLK + 1):
            if b > 0:
                nc.sync.dma_start(out=H1[16:17, 4 * BC:5 * BC],
                                  in_=dram["ones_row"][0:BC].rearrange("(o a) -> o a", o=1))
                xgates(1, b - 1, wih[1], G2)
            if b < NBLK:
                cur_M = mask_block(b)
            pieces = []
            if b + 1 < NBLK:
                obs_next = enc_dma(b + 1)
                fslot = (b + 1) % 2
                for m in range(4):
                    pieces.append(lambda m=m, b1=b + 1, ot=obs_next, fs=fslot:
                                  enc_piece(b1, ot, m, featl[fs]))
                for g in range(5):
                    pieces.append(lambda g=g, b1=b + 1, fs=fslot:
                                  xg_grp(0, b1, wih[0], G1l[fs], featl[fs], g))
            npieces = len(pieces)
            for t in range(Tb):
                if b > 0:
                    rec_step(1, b - 1, t, G2, prev_M, H2)
                if b < NBLK:
                    rec_step(0, b, t, G1l[b % 2], cur_M, H1)
                while pieces and len(pieces) > npieces * (Tb - 1 - t) // Tb:
                    pieces.pop(0)()
            if b > 0:
                heads(b - 1)
            if b < NBLK:
                prev_M = cur_M


_NC_CACHE = {}

def _get_nc():
    if "nc" not in _NC_CACHE:
        _NC_CACHE["nc"] = _build_nc()
    return _NC_CACHE["nc"]


def _pad_gate_T(W):
    """(2112, 528) -> transposed padded (Dp, Gp) = W^T with per-gate row padding."""
    Wp = np.zeros((Gp, Dp), np.float32)
    for dst, srcg in enumerate(GATE_ORDER):
        Wp[dst * 640:dst * 640 + D, :D] = W[srcg * D:(srcg + 1) * D, :]
    return np.ascontiguousarray(Wp.T)


def _pad_gate_b(b):
    bp = np.zeros((Gp,), np.float32)
    for dst, srcg in enumerate(GATE_ORDER):
        bp[dst * 640:dst * 640 + D] = b[srcg * D:(srcg + 1) * D]
    return bp


def kernel(obs, last_action, reward, terminated,
           enc_w, enc_b,
           w_ih0, w_hh0, b_ih0, b_hh0,
           w_ih1, w_hh1, b_ih1, b_hh1,
           pol_w, pol_b, base_w, base_b):
    obs = np.asarray(obs, np.float32)
    last_action = np.asarray(last_action, np.int32)
    reward = np.asarray(reward, np.float32)
    terminated = np.asarray(terminated, np.int32)

    nc = _get_nc()

    # shared (replicated) weight tensors
    shared = {
        "enc_wT": np.ascontiguousarray(np.asarray(enc_w, np.float32).T).astype(BF),
        "enc_b": np.asarray(enc_b, np.float32),
        "headWT": None,
        "headb": np.concatenate([np.asarray(pol_b, np.float32),
                                 np.asarray(base_b, np.float32)]),
    }
    hw = np.zeros((Dp, 16), np.float32)
    hw[:D, :A] = np.asarray(pol_w, np.float32).T
    hw[:D, A] = np.asarray(base_w, np.float32)[0]
    shared["headWT"] = hw.astype(BF)
    for l, (wi, wh, bi, bh) in enumerate([
            (w_ih0, w_hh0, b_ih0, b_hh0), (w_ih1, w_hh1, b_ih1, b_hh1)]):
        wihTp = _pad_gate_T(np.asarray(wi, np.float32)).astype(np.float32)
        bias = _pad_gate_b(np.asarray(bi, np.float32) + np.asarray(bh, np.float32))
        wihTp[D, :] = bias          # constant-1.0 input row carries the bias
        shared[f"wihT{l}"] = wihTp.astype(BF)
        shared[f"whhT{l}"] = _pad_gate_T(np.asarray(wh, np.float32)).astype(BF)

    obs_bf = obs.astype(BF)
    ndm_full = (terminated == 0).astype(np.float32)           # (T, B)

    in_maps = []
    for c in range(NC_):
        b0 = c * BL
        sl = slice(b0, b0 + BL)
        obsT = np.ascontiguousarray(
            obs_bf[:, sl, :].reshape(TB, OBS).T)              # (OBS, TB) bf16
        ndm = np.concatenate([ndm_full[:, sl],
                              np.ones((1, BL), np.float32)], axis=0).reshape(-1)
        m = {
            "obsT": obsT,
            "rew": np.ascontiguousarray(reward[:, sl]).reshape(-1),
            "act": np.ascontiguousarray(last_action[:, sl]).reshape(-1),
            "ndm": np.ascontiguousarray(ndm),
            "ones_row": np.ones((TB,), BF),
        }
        m.update(shared)
        in_maps.append(m)

    res = run_bass_kernel_spmd(nc, in_maps, list(range(NC_)))

    pol = np.zeros((T, B, A), np.float32)
    bsl = np.zeros((T, B), np.float32)
    aout = np.zeros((T, B), np.int32)
    for c in range(NC_):
        r = res.results[c]
        sl = slice(c * BL, (c + 1) * BL)
        pol[:, sl, :] = np.asarray(r["pol"], np.float32).reshape(T, BL, A)
        bsl[:, sl] = np.asarray(r["bsl"], np.float32).reshape(T, BL)
        aout[:, sl] = np.asarray(r["aout"], np.int32).reshape(T, BL)
    return pol, bsl, aout
